# revision 18
# baseline (speedup 1.0000x reference)
"""Trainium2 Bass kernel for nn_CustomLoss (argmax-distance weighted loss).

reference:
    arg = argmax(target, axis=1)              # [B]
    delta = distance[arg]                     # [B]
    err = |distance[None,:] - delta[:,None]| + 1
    loss = sum((output - target) * err) / B

Algorithm (bucket matmul, data-parallel over 8 NeuronCores):
  err[b,:] is one of 5 constant rows W[a,:] = 1 + |dist - dist[a]|, selected
  by a = argmax(target[b]).  So with the one-hot E[b,a] = [argmax==a]:
      loss*B = sum_{a,c} W[a,c] * P[a,c],   P = E^T @ (O - T)   (5x5)
  P is accumulated on TensorE: rows land on partitions (128) x free dim
  (g,c); per 16-row block l, matmul(lhsT=E_blk[128,80], rhs=[T|O]_blk
  [128,2,80]) accumulates out[(l,a),(s,l',c)] in PSUM over the whole shard;
  only the l==l' diagonal 5x5 blocks are meaningful and the host sums them.

Schedule: ALL loads go through the single sync HWDGE ring, whose FIFO order
enforces the phasing the Tile scheduler would otherwise break: 8 t tiles
first (they feed the ScalarE cast + VectorE max/is_ge chains), then the o
tiles (half-DMAs; the last tile in quarters), so the tail of the HBM stream
feeds only a short cast + matmul chain.  No SWDGE involvement (any SWDGE
traffic slows DMA engine 15 ~15% via its descriptor-ring port contention
and every DMA completion waits on all 16 engines).  ScalarE casts t and the
o tiles 0..6; the last o tile is cast by VectorE, which is idle by then
while ScalarE still lags the stream.  max/is_ge are split into tile halves
so matmuls chase E at half-tile granularity.  Output per core: [80, 160]
f32; host sums the block-diagonal 5x5s.
"""

from contextlib import ExitStack

import numpy as np

P = 128
C = 5
DIST = (-0.5, -0.34, 0.0, 0.34, 0.5)
B = 4194304
NCORES = 8
ROWS_PER_CORE = B // NCORES  # 524288
G = 512                      # rows per partition per tile
NTILES = ROWS_PER_CORE // (P * G)  # 8
FREE = G * C                 # 2560
HG = G // 2                  # 256
HFREE = FREE // 2            # 1280
BLK = 16                     # rows-per-partition per matmul block
BLKC = BLK * C               # 80 = lhsT columns = psum partitions
NBLK = G // BLK              # 32 matmul blocks per tile
MOUT = BLKC                  # 80
NOUT = 2 * BLKC              # 160

_CACHE = {}


def _build_nc():
    import concourse.bacc as bacc
    import concourse.mybir as mybir
    import concourse.tile as tile

    F32 = mybir.dt.float32
    BF16 = mybir.dt.bfloat16

    nc = bacc.Bacc(target_bir_lowering=False)

    t_in = nc.declare_dram_parameter("t", [ROWS_PER_CORE, C], F32, isOutput=False)
    o_in = nc.declare_dram_parameter("o", [ROWS_PER_CORE, C], F32, isOutput=False)
    out = nc.declare_dram_parameter("out", [MOUT, NOUT], F32, isOutput=True)

    # row = n*(P*G) + p*G + g ; per-partition data is contiguous in DRAM
    t_tiled = t_in.rearrange("(n p g) c -> n p (g c)", p=P, g=G)
    o_tiled = o_in.rearrange("(n p g) c -> n p (g c)", p=P, g=G)

    with ExitStack() as ctx:
        tc = ctx.enter_context(tile.TileContext(nc))
        pool = ctx.enter_context(tc.tile_pool(name="work", bufs=2))
        psp = ctx.enter_context(tc.tile_pool(name="ps", bufs=1, space="PSUM"))
        outp = ctx.enter_context(tc.tile_pool(name="outp", bufs=1))
        ps = psp.tile([MOUT, NOUT], F32)

        # phase 1: the 8 t tiles head the HWDGE ring FIFO
        tts = []
        for k in range(NTILES):
            tt = pool.tile([P, FREE], F32, tag="t", name="tt", bufs=6)
            nc.sync.dma_start(tt[:, :], t_tiled[k])
            tts.append(tt)

        # phase 2: o tiles (f32) queue behind them on the same ring (any SWDGE
        # traffic would slow DMA engine 15 ~15% and stretch the whole stream);
        # the last tile lands in quarters so its tail chases at ~1us grain
        ofs = []
        for k in range(NTILES):
            of = pool.tile([P, FREE], F32, tag="of", name="of", bufs=3)
            nq = 4 if k == NTILES - 1 else 2
            q = FREE // nq
            for h in range(nq):
                nc.sync.dma_start(
                    of[:, h * q : (h + 1) * q],
                    o_tiled[k][:, h * q : (h + 1) * q],
                )
            ofs.append(of)

        # to_k = [t_bf16 | o_bf16] side by side so one matmul streams both
        tos = [
            pool.tile([P, 2 * FREE], BF16, tag="to", name="to", bufs=NTILES - 1)
            for _ in range(NTILES)
        ]

        for k in range(NTILES):
            tt, of, to = tts[k], ofs[k], tos[k]
            nc.scalar.copy(to[:, 0:FREE], tt[:, :])  # ACT cast f32->bf16
            if k < NTILES - 1:  # ACT cast f32->bf16, chasing the o half-DMAs
                for h in range(2):
                    nc.scalar.copy(
                        to[:, FREE + h * HFREE : FREE + (h + 1) * HFREE],
                        of[:, h * HFREE : (h + 1) * HFREE],
                    )
            else:
                # last tile: VectorE is free by now and ACT lags the stream;
                # cast the four o quarters on DVE (2x fp32 single-src mode)
                q = FREE // 4
                for h in range(4):
                    nc.vector.tensor_copy(
                        to[:, FREE + h * q : FREE + (h + 1) * q],
                        of[:, h * q : (h + 1) * q],
                    )

            E = pool.tile([P, FREE], BF16, tag="E", name="E", bufs=NTILES)
            for h in range(2):  # half-tile max/is_ge so matmuls chase E
                tv = tt[:, h * HFREE : (h + 1) * HFREE].rearrange(
                    "p (g c) -> p g c", c=C
                )
                m = pool.tile([P, HG], F32, tag="m", name="m", bufs=2)
                nc.vector.tensor_reduce(
                    m[:, :], tv, axis=mybir.AxisListType.X, op=mybir.AluOpType.max
                )
                nc.vector.tensor_tensor(
                    E[:, h * HFREE : (h + 1) * HFREE].rearrange(
                        "p (g c) -> p g c", c=C
                    ),
                    tv,
                    m[:, :].to_broadcast([P, HG, C]),
                    op=mybir.AluOpType.is_ge,
                )

            tov = to[:, :].rearrange("p (s f) -> p s f", s=2)
            for blk in range(NBLK):
                first = k == 0 and blk == 0
                last = k == NTILES - 1 and blk == NBLK - 1
                sl = slice(blk * BLKC, (blk + 1) * BLKC)
                nc.tensor.matmul(
                    ps[:, :], E[:, sl], tov[:, :, sl], start=first, stop=last
                )

        res = outp.tile([MOUT, NOUT], F32)
        nc.scalar.copy(res[:, :], ps[:, :])
        nc.sync.dma_start(out[:, :], res[:, :])
    nc.finalize()
    return nc


def _get_nc():
    if "nc" not in _CACHE:
        _CACHE["nc"] = _build_nc()
    return _CACHE["nc"]


def _reduce_loss(results):
    """results: iterable of per-core out arrays [80, 160] f32 -> loss."""
    dist = np.asarray(DIST, np.float64)
    W = 1.0 + np.abs(dist[None, :] - dist[:, None])  # [a, c]
    total = 0.0
    for arr in results:
        r = arr.astype(np.float64).reshape(BLK, C, 2, BLK, C)  # (l,a,s,l',c)
        Pm = np.einsum("dasdc->sac", r)  # diag over l; [2(s=t,o), 5, 5]
        total += float((W * (Pm[1] - Pm[0])).sum())
    return total / B


def kernel(output, target, distance, _want_results=False):
    from concourse.bass_utils import run_bass_kernel_spmd

    output = np.asarray(output, dtype=np.float32)
    target = np.asarray(target, dtype=np.float32)
    distance = np.asarray(distance, dtype=np.float32)
    assert output.shape == (B, C) and target.shape == (B, C)
    assert np.allclose(distance, np.asarray(DIST, np.float32)), distance

    nc = _get_nc()
    o_sh = output.reshape(NCORES, ROWS_PER_CORE, C)
    t_sh = target.reshape(NCORES, ROWS_PER_CORE, C)
    in_maps = [
        {"t": np.ascontiguousarray(t_sh[i]), "o": np.ascontiguousarray(o_sh[i])}
        for i in range(NCORES)
    ]
    res = run_bass_kernel_spmd(nc, in_maps, core_ids=list(range(NCORES)))
    loss = np.array(_reduce_loss(r["out"] for r in res.results), dtype=np.float32)
    if _want_results:
        return loss, res
    return loss


# revision 19
# speedup vs baseline: 1.0116x; 1.0116x over previous
"""Trainium2 Bass kernel for nn_CustomLoss (argmax-distance weighted loss).

reference:
    arg = argmax(target, axis=1)              # [B]
    delta = distance[arg]                     # [B]
    err = |distance[None,:] - delta[:,None]| + 1
    loss = sum((output - target) * err) / B

Algorithm (bucket matmul, data-parallel over 8 NeuronCores):
  err[b,:] is one of 5 constant rows W[a,:] = 1 + |dist - dist[a]|, selected
  by a = argmax(target[b]).  So with the one-hot E[b,a] = [argmax==a]:
      loss*B = sum_{a,c} W[a,c] * P[a,c],   P = E^T @ (O - T)   (5x5)
  P is accumulated on TensorE: rows land on partitions (128) x free dim
  (g,c); per 16-row block l, matmul(lhsT=E_blk[128,80], rhs=[T|O]_blk
  [128,2,80]) accumulates out[(l,a),(s,l',c)] in PSUM over the whole shard;
  only the l==l' diagonal 5x5 blocks are meaningful and the host sums them.

Schedule: ALL loads go through the single sync HWDGE ring, whose FIFO order
enforces the phasing the Tile scheduler would otherwise break: 8 t tiles
first (they feed the ScalarE cast + VectorE max/is_ge chains), then the o
tiles (half-DMAs; the last tile in quarters), so the tail of the HBM stream
feeds only a short cast + matmul chain.  No SWDGE involvement (any SWDGE
traffic slows DMA engine 15 ~15% via its descriptor-ring port contention
and every DMA completion waits on all 16 engines).  ScalarE casts t and the
o tiles 0..6; the last o tile is cast by VectorE, which is idle by then
while ScalarE still lags the stream.  max/is_ge are split into tile halves
so matmuls chase E at half-tile granularity.  Output per core: [80, 160]
f32; host sums the block-diagonal 5x5s.
"""

from contextlib import ExitStack

import numpy as np

P = 128
C = 5
DIST = (-0.5, -0.34, 0.0, 0.34, 0.5)
B = 4194304
NCORES = 8
ROWS_PER_CORE = B // NCORES  # 524288
G = 512                      # rows per partition per tile
NTILES = ROWS_PER_CORE // (P * G)  # 8
FREE = G * C                 # 2560
HG = G // 2                  # 256
HFREE = FREE // 2            # 1280
BLK = 16                     # rows-per-partition per matmul block
BLKC = BLK * C               # 80 = lhsT columns = psum partitions
NBLK = G // BLK              # 32 matmul blocks per tile
MOUT = BLKC                  # 80
NOUT = 2 * BLKC              # 160

_CACHE = {}


def _build_nc():
    import concourse.bacc as bacc
    import concourse.mybir as mybir
    import concourse.tile as tile

    F32 = mybir.dt.float32
    BF16 = mybir.dt.bfloat16

    nc = bacc.Bacc(target_bir_lowering=False)

    t_in = nc.declare_dram_parameter("t", [ROWS_PER_CORE, C], F32, isOutput=False)
    o_in = nc.declare_dram_parameter("o", [ROWS_PER_CORE, C], F32, isOutput=False)
    out = nc.declare_dram_parameter("out", [MOUT, NOUT], F32, isOutput=True)

    # row = n*(P*G) + p*G + g ; per-partition data is contiguous in DRAM
    t_tiled = t_in.rearrange("(n p g) c -> n p (g c)", p=P, g=G)
    o_tiled = o_in.rearrange("(n p g) c -> n p (g c)", p=P, g=G)

    with ExitStack() as ctx:
        tc = ctx.enter_context(tile.TileContext(nc))
        pool = ctx.enter_context(tc.tile_pool(name="work", bufs=2))
        psp = ctx.enter_context(tc.tile_pool(name="ps", bufs=1, space="PSUM"))
        outp = ctx.enter_context(tc.tile_pool(name="outp", bufs=1))
        ps = psp.tile([MOUT, NOUT], F32)

        # phase 1: the 8 t tiles head the HWDGE ring FIFO
        tts = []
        for k in range(NTILES):
            tt = pool.tile([P, FREE], F32, tag="t", name="tt", bufs=5)
            nc.sync.dma_start(tt[:, :], t_tiled[k])
            tts.append(tt)

        # phase 2: o tiles (f32) queue behind them on the same ring (any SWDGE
        # traffic would slow DMA engine 15 ~15% and stretch the whole stream);
        # the last tile lands in quarters so its tail chases at ~1us grain
        ofs = []
        for k in range(NTILES):
            of = pool.tile([P, FREE], F32, tag="of", name="of", bufs=3)
            nq = 4 if k == NTILES - 1 else 2
            q = FREE // nq
            for h in range(nq):
                nc.sync.dma_start(
                    of[:, h * q : (h + 1) * q],
                    o_tiled[k][:, h * q : (h + 1) * q],
                )
            ofs.append(of)

        # to_k = [t_bf16 | o_bf16] side by side so one matmul streams both
        tos = [
            pool.tile([P, 2 * FREE], BF16, tag="to", name="to", bufs=NTILES)
            for _ in range(NTILES)
        ]

        for k in range(NTILES):
            tt, of, to = tts[k], ofs[k], tos[k]
            nc.scalar.copy(to[:, 0:FREE], tt[:, :])  # ACT cast f32->bf16
            if k < NTILES - 1:  # ACT cast f32->bf16, chasing the o half-DMAs
                for h in range(2):
                    nc.scalar.copy(
                        to[:, FREE + h * HFREE : FREE + (h + 1) * HFREE],
                        of[:, h * HFREE : (h + 1) * HFREE],
                    )
            else:
                # last tile: VectorE is free by now and ACT lags the stream;
                # cast the four o quarters on DVE (2x fp32 single-src mode)
                q = FREE // 4
                for h in range(4):
                    nc.vector.tensor_copy(
                        to[:, FREE + h * q : FREE + (h + 1) * q],
                        of[:, h * q : (h + 1) * q],
                    )

            E = pool.tile([P, FREE], BF16, tag="E", name="E", bufs=NTILES)
            for h in range(2):  # half-tile max/is_ge so matmuls chase E
                tv = tt[:, h * HFREE : (h + 1) * HFREE].rearrange(
                    "p (g c) -> p g c", c=C
                )
                m = pool.tile([P, HG], F32, tag="m", name="m", bufs=2)
                nc.vector.tensor_reduce(
                    m[:, :], tv, axis=mybir.AxisListType.X, op=mybir.AluOpType.max
                )
                nc.vector.tensor_tensor(
                    E[:, h * HFREE : (h + 1) * HFREE].rearrange(
                        "p (g c) -> p g c", c=C
                    ),
                    tv,
                    m[:, :].to_broadcast([P, HG, C]),
                    op=mybir.AluOpType.is_ge,
                )

            tov = to[:, :].rearrange("p (s f) -> p s f", s=2)
            for blk in range(NBLK):
                first = k == 0 and blk == 0
                last = k == NTILES - 1 and blk == NBLK - 1
                sl = slice(blk * BLKC, (blk + 1) * BLKC)
                nc.tensor.matmul(
                    ps[:, :], E[:, sl], tov[:, :, sl], start=first, stop=last
                )

        res = outp.tile([MOUT, NOUT], F32)
        nc.scalar.copy(res[:, :], ps[:, :])
        nc.sync.dma_start(out[:, :], res[:, :])
    nc.finalize()
    return nc


def _get_nc():
    if "nc" not in _CACHE:
        _CACHE["nc"] = _build_nc()
    return _CACHE["nc"]


def _reduce_loss(results):
    """results: iterable of per-core out arrays [80, 160] f32 -> loss."""
    dist = np.asarray(DIST, np.float64)
    W = 1.0 + np.abs(dist[None, :] - dist[:, None])  # [a, c]
    total = 0.0
    for arr in results:
        r = arr.astype(np.float64).reshape(BLK, C, 2, BLK, C)  # (l,a,s,l',c)
        Pm = np.einsum("dasdc->sac", r)  # diag over l; [2(s=t,o), 5, 5]
        total += float((W * (Pm[1] - Pm[0])).sum())
    return total / B


def kernel(output, target, distance, _want_results=False):
    from concourse.bass_utils import run_bass_kernel_spmd

    output = np.asarray(output, dtype=np.float32)
    target = np.asarray(target, dtype=np.float32)
    distance = np.asarray(distance, dtype=np.float32)
    assert output.shape == (B, C) and target.shape == (B, C)
    assert np.allclose(distance, np.asarray(DIST, np.float32)), distance

    nc = _get_nc()
    o_sh = output.reshape(NCORES, ROWS_PER_CORE, C)
    t_sh = target.reshape(NCORES, ROWS_PER_CORE, C)
    in_maps = [
        {"t": np.ascontiguousarray(t_sh[i]), "o": np.ascontiguousarray(o_sh[i])}
        for i in range(NCORES)
    ]
    res = run_bass_kernel_spmd(nc, in_maps, core_ids=list(range(NCORES)))
    loss = np.array(_reduce_loss(r["out"] for r in res.results), dtype=np.float32)
    if _want_results:
        return loss, res
    return loss
